# revision 19
# baseline (speedup 1.0000x reference)
"""Trainium2 Bass kernel for the GNN message-passing Convolution problem.

Strategy (8 NeuronCores, SPMD):
  - Host: sort edges by destination node; shard destination nodes 8 ways
    (6250/core); within a core, group edges into bins of 128 consecutive
    dst nodes, padded to a uniform number of 128-edge tiles per bin so the
    single SPMD program works for every core.
  - Per-exec dispatch in this environment carries a large fixed cost per
    input tensor, so everything is packed into THREE device tensors per
    core: a [32, W] bf16 blob (weights + node features + edge features),
    a [16, W] bf16 blob (node attrs + attr-side weights), and one int32
    tensor holding (dst_offset << 16 | src_index) per edge slot. Output
    is returned as bf16 and upcast on host.
  - Device, per core:
      Phase N: node linears x = in*attr*W_lin1, s = in*attr*W_sc computed in
        "transposed land" (features on partitions, nodes on free dim) with
        PE matmuls; x rows are transposed back and written to DRAM.
      AllGather x shards -> full x table (needed for src gathers).
      Phase E: per 128-edge tile: radial MLP on PE (bf16), indirect-DMA
        gather of x[src], bilinear message on DVE, one-hot scatter matmul
        accumulating each bin's [128 nodes x 256] in PSUM -> SBUF slab.
      Phase F: per bin: lin2 (agg*attr*W_lin2) + self-connection, DMA out.
  - Host: concatenate the 8 node shards.
"""

import math
import sys

import numpy as np

if "/opt/trn_rl_repo" not in sys.path:
    sys.path.insert(0, "/opt/trn_rl_repo")

import concourse.bacc as bacc
import concourse.bass as bass
import concourse.mybir as mybir
from concourse.bass import IndirectOffsetOnAxis
from concourse.bass_utils import run_bass_kernel_spmd
from concourse.masks import make_identity
from concourse.tile import TileContext

F32 = mybir.dt.float32
BF16 = mybir.dt.bfloat16
I32 = mybir.dt.int32

NCORES = 8
C_S = math.sin(math.pi / 8.0)
C_X = math.cos(math.pi / 8.0)
INV_SQRT_NEI = 1.0 / math.sqrt(8.0)

# column offsets inside the [32, W32] bf16 blob
_O_W1 = 0          # fc_w1            [32, 64]
_O_W2 = 64         # fc_w2 packed     [64, 512] as 2x[32, 512]
_O_B2 = 1088       # fc_b2 packed     [1, 512] (row 0)
_O_B1 = 1600       # fc_b1            [1, 64]  (row 0)
_O_SSEL = 1664     # ssel             [128, 128] as 4x[32, 128]
_O_W1X = 2176      # W_lin1 flat      [8, 128]
_O_W1S = 2304      # W_sc*sin flat    [8, 128]
_O_IN = 2432       # node_input^T     [32, npad]
# efT [32, s_total] starts at _O_IN + npad, then the [16, npad+192] attr
# blob folded as [32, (npad+192)/2], then epackT int32 bytes as bf16
# [32, (s_total/128)*128*4/2/32] (4 partition-groups of 32 rows).


def _to_bf16(x):
    import ml_dtypes

    return np.asarray(x, np.float32).astype(ml_dtypes.bfloat16)


# ---------------------------------------------------------------- host prep
def _host_prep(inputs, ns, nbin):
    """Build per-core input maps. ns = dst nodes per core, nbin = node bin size."""
    node_input = np.ascontiguousarray(inputs["node_input"], np.float32)   # [N,4,8]
    node_attr = np.ascontiguousarray(inputs["node_attr"], np.float32)    # [N,16]
    edge_feat = np.ascontiguousarray(inputs["edge_features"], np.float32)  # [E,32]
    W_sc = np.asarray(inputs["W_sc"], np.float32)      # [8,16,8]
    W_lin1 = np.asarray(inputs["W_lin1"], np.float32)  # [8,16,8]
    W_lin2 = np.asarray(inputs["W_lin2"], np.float32)  # [8,16,8]
    fc_w1 = np.asarray(inputs["fc_w1"], np.float32)    # [32,64]
    fc_b1 = np.asarray(inputs["fc_b1"], np.float32)    # [64]
    fc_w2 = np.asarray(inputs["fc_w2"], np.float32)    # [64,512]
    fc_b2 = np.asarray(inputs["fc_b2"], np.float32)    # [512]
    src = np.asarray(inputs["edge_src"], np.int32)
    dst = np.asarray(inputs["edge_dst"], np.int32)

    n = node_input.shape[0]
    nb = (ns + nbin - 1) // nbin        # bins per core
    npad = nb * nbin                    # padded nodes per core

    # --- per-core edge binning (uniform tiles/bin across all cores) ---
    core_of = dst // ns
    local_dst = dst - core_of * ns
    bin_of = local_dst // nbin
    counts = np.zeros((NCORES, nb), np.int64)
    np.add.at(counts, (core_of, bin_of), 1)
    tiles_per_bin = int(-(-counts.max() // 128))
    slots_per_bin = tiles_per_bin * 128
    s_total = nb * slots_per_bin

    # slot index for every edge: sort by (core, bin), place sequentially in bin
    order = np.lexsort((dst,))  # stable sort by dst => sorted by (core,bin)
    grp = core_of[order] * nb + bin_of[order]
    first = np.r_[True, grp[1:] != grp[:-1]]
    idx_of_first = np.maximum.accumulate(np.where(first, np.arange(len(grp)), 0))
    rank_in_bin = np.arange(len(grp)) - idx_of_first

    # --- shared weight blob pieces (bf16) ---
    # fc_w2 cols are (d, i, o); permute to (i, o, d) so per-i slices are flat.
    w2p = fc_w2.reshape(64, 8, 8, 8).transpose(0, 2, 3, 1).reshape(2, 32, 512)
    b2p = fc_b2.reshape(8, 8, 8).transpose(1, 2, 0).reshape(512)
    w1x = W_lin1.reshape(8, 128)                                # [(i),(a,j)]
    w1s = (W_sc * C_S).reshape(8, 128)                          # [(i),(a,o)]
    ssel = np.zeros((16, 8, 4, 4, 8), np.float32)
    for a in range(16):
        for j in range(8):
            for c in range(4):
                ssel[a, j, c, c, j] = 1.0
    ssel = ssel.reshape(128, 128).reshape(4, 32, 128)           # [(a,j),(c,c',j')]
    w2lr = (W_lin2 * (C_X * INV_SQRT_NEI)).transpose(1, 0, 2).reshape(16, 64)
    repa = np.zeros((16, 16, 8), np.float32)
    for a in range(16):
        repa[a, a, :] = 1.0
    repa = repa.reshape(16, 128)

    ntile = s_total // 128
    OA16 = _O_IN + npad + s_total          # attr blob [32, (npad+192)//2]
    wa16 = (npad + 192) // 2
    OEP = OA16 + wa16                      # epack bytes [32, 8*ntile]
    W32 = OEP + 8 * ntile
    head = np.zeros((32, _O_IN), np.float32)
    head[:, _O_W1 : _O_W1 + 64] = fc_w1
    head[:, _O_W2 : _O_W2 + 512] = w2p[0]
    head[:, _O_W2 + 512 : _O_W2 + 1024] = w2p[1]
    head[0, _O_B2 : _O_B2 + 512] = b2p
    head[0, _O_B1 : _O_B1 + 64] = fc_b1
    for q in range(4):
        head[:, _O_SSEL + q * 128 : _O_SSEL + (q + 1) * 128] = ssel[q]
    head[:8, _O_W1X : _O_W1X + 128] = w1x
    head[:8, _O_W1S : _O_W1S + 128] = w1s

    tail16 = np.zeros((16, 192), np.float32)
    tail16[:, :128] = repa
    tail16[:, 128:192] = w2lr
    head_bf = _to_bf16(head)

    in_maps = []
    for k in range(NCORES):
        lo = k * ns
        mask = core_of[order] == k
        slot = bin_of[order][mask] * slots_per_bin + rank_in_bin[mask]
        eidx = order[mask]
        lb = dst[eidx] - lo

        efT = np.zeros((32, s_total), np.float32)
        efT[:, slot] = edge_feat[eidx].T
        sv = src[eidx]
        # epack: (dst offset within bin) << 16 | global padded src index
        ep_flat = np.full(s_total, 255 << 16, np.int32)
        ep_flat[slot] = ((lb % nbin).astype(np.int32) << 16) | (
            (sv // ns) * npad + (sv % ns)
        )
        epackT = np.ascontiguousarray(ep_flat.reshape(-1, 128).T)  # [128, ntile]

        sl = slice(lo, lo + ns)
        import ml_dtypes

        blob32 = np.zeros((32, W32), ml_dtypes.bfloat16)
        blob32[:, : _O_IN] = head_bf
        blob32[:, _O_IN : _O_IN + ns] = _to_bf16(node_input[sl].reshape(ns, 32).T)
        blob32[:, _O_IN + npad : OA16] = _to_bf16(efT)
        a16 = np.zeros((16, npad + 192), np.float32)
        a16[:, :ns] = node_attr[sl].T
        a16[:, npad:] = tail16
        a16 = _to_bf16(a16)
        blob32[:16, OA16:OEP] = a16[:, :wa16]
        blob32[16:, OA16:OEP] = a16[:, wa16:]
        # raw int32 bytes viewed as bf16: partition-group q of 32 rows
        epv = epackT.view(np.uint16).reshape(4, 32, 2 * ntile).view(
            ml_dtypes.bfloat16
        )
        for q in range(4):
            blob32[:, OEP + q * 2 * ntile : OEP + (q + 1) * 2 * ntile] = epv[q]

        in_maps.append({"blob32": blob32})
    return in_maps, tiles_per_bin, nb, npad, s_total


# ---------------------------------------------------------------- device
def _build(tiles_per_bin, nb, npad, s_total, ns):
    T = tiles_per_bin
    nc = bacc.Bacc("TRN2", debug=False, num_devices=NCORES)

    ntile = s_total // 128
    OA16 = _O_IN + npad + s_total
    wa16 = (npad + 192) // 2
    OEP = OA16 + wa16
    W32 = OEP + 8 * ntile
    d_b32 = nc.dram_tensor("blob32", [32, W32], BF16, kind="ExternalInput").ap()
    d_out = nc.dram_tensor("out", [npad, 256], BF16, kind="ExternalOutput").ap()

    OEF = _O_IN + npad
    mult = mybir.AluOpType.mult
    addop = mybir.AluOpType.add

    with TileContext(nc) as tc:
        with (
            tc.tile_pool(name="const", bufs=1) as const,
            tc.tile_pool(name="dram", bufs=1, space="DRAM") as dram,
        ):
            # persistent SBUF state
            ident = const.tile([128, 128], F32)
            make_identity(nc, ident[:])
            w1_sb = const.tile([32, 64], BF16)
            nc.sync.dma_start(out=w1_sb[:], in_=d_b32[:, _O_W1 : _O_W1 + 64])
            w2p64_sb = const.tile([64, 512], BF16)
            for q in range(2):
                nc.sync.dma_start(
                    out=w2p64_sb[q * 32 : (q + 1) * 32, :],
                    in_=d_b32[:, _O_W2 + q * 512 : _O_W2 + (q + 1) * 512],
                )
            b2p_sb = const.tile([1, 512], BF16)
            nc.sync.dma_start(out=b2p_sb[:], in_=d_b32[0:1, _O_B2 : _O_B2 + 512])
            b1_sb = const.tile([1, 64], BF16)
            nc.sync.dma_start(out=b1_sb[:], in_=d_b32[0:1, _O_B1 : _O_B1 + 64])
            ones_sb = const.tile([1, 128], BF16)
            nc.vector.memset(ones_sb[:], 1.0)
            ssel_sb = const.tile([128, 128], BF16)
            for q in range(4):
                nc.sync.dma_start(
                    out=ssel_sb[q * 32 : (q + 1) * 32, :],
                    in_=d_b32[:, _O_SSEL + q * 128 : _O_SSEL + (q + 1) * 128],
                )
            w1x_sb = const.tile([8, 128], BF16)
            nc.sync.dma_start(out=w1x_sb[:], in_=d_b32[0:8, _O_W1X : _O_W1X + 128])
            w1s_sb = const.tile([8, 128], BF16)
            nc.sync.dma_start(out=w1s_sb[:], in_=d_b32[0:8, _O_W1S : _O_W1S + 128])
            a16_sb = const.tile([16, npad + 192], BF16)
            nc.sync.dma_start(out=a16_sb[:, :wa16], in_=d_b32[0:16, OA16:OEP])
            nc.sync.dma_start(out=a16_sb[:, wa16:], in_=d_b32[16:32, OA16:OEP])
            attrT_sb = a16_sb[:, :npad]
            repa_sb = a16_sb[:, npad : npad + 128]
            w2lr_sb = a16_sb[:, npad + 128 : npad + 192]
            iota_sb = const.tile([128, 128], F32)
            nc.gpsimd.iota(
                iota_sb[:],
                pattern=[[1, 128]],
                base=0,
                channel_multiplier=0,
                allow_small_or_imprecise_dtypes=True,
            )
            epraw_sb = const.tile([128, 2 * ntile], BF16)
            for q in range(4):
                nc.sync.dma_start(
                    out=epraw_sb[q * 32 : (q + 1) * 32, :],
                    in_=d_b32[:, OEP + q * 2 * ntile : OEP + (q + 1) * 2 * ntile],
                )
            esrc_sb = const.tile([128, ntile], I32)
            nc.vector.tensor_scalar(
                out=esrc_sb[:],
                in0=epraw_sb[:].bitcast(I32),
                scalar1=0xFFFF,
                scalar2=None,
                op0=mybir.AluOpType.bitwise_and,
            )
            dstoi_sb = const.tile([128, ntile], I32)
            nc.vector.tensor_scalar(
                out=dstoi_sb[:],
                in0=epraw_sb[:].bitcast(I32),
                scalar1=16,
                scalar2=None,
                op0=mybir.AluOpType.logical_shift_right,
            )
            dstoff_sb = const.tile([128, ntile], F32)
            nc.scalar.copy(out=dstoff_sb[:], in_=dstoi_sb[:])
            sT_sb = const.tile([32, npad], F32)
            slab = const.tile([128, nb * 256], F32)

            x_shard = dram.tile([npad, 32], F32)
            x_full = dram.tile([NCORES * npad, 32], F32, addr_space="Shared")

            # ---------------- phase N: node linears ----------------
            chunks = []
            base = 0
            while base < npad:
                cw = min(512, npad - base)
                chunks.append((base, cw))
                base += cw
            with (
                tc.tile_pool(name="n1", bufs=3) as pn,
                tc.tile_pool(name="n1ps", bufs=2, space="PSUM") as pnps,
            ):
                for base, cw in chunks:
                    inT_cs = []
                    for c in range(4):
                        t = pn.tile([8, cw], BF16, tag=f"inT{c}")
                        nc.sync.dma_start(
                            out=t[:],
                            in_=d_b32[
                                c * 8 : (c + 1) * 8, _O_IN + base : _O_IN + base + cw
                            ],
                        )
                        inT_cs.append(t)
                    atr_ps = pnps.tile([128, cw], F32, tag="atrp", bufs=1)
                    nc.tensor.matmul(
                        out=atr_ps[:],
                        lhsT=repa_sb[:],
                        rhs=attrT_sb[:, base : base + cw],
                        start=True,
                        stop=True,
                    )
                    atr_sb = pn.tile([128, cw], F32, tag="atr")
                    nc.scalar.copy(out=atr_sb[:], in_=atr_ps[:])
                    xT_ps = pnps.tile([32, cw], F32, tag="xT", bufs=1)
                    sT_ps = pnps.tile([32, cw], F32, tag="sT", bufs=1)
                    for c in range(4):
                        rhs = inT_cs[c][:]
                        u_ps = pnps.tile([128, cw], F32, tag="u")
                        nc.tensor.matmul(
                            out=u_ps[:], lhsT=w1x_sb[:], rhs=rhs, start=True, stop=True
                        )
                        pr_sb = pn.tile([128, cw], BF16, tag="pr")
                        nc.vector.tensor_tensor(
                            out=pr_sb[:], in0=u_ps[:], in1=atr_sb[:], op=mult
                        )
                        nc.tensor.matmul(
                            out=xT_ps[:],
                            lhsT=ssel_sb[:, c * 32 : (c + 1) * 32],
                            rhs=pr_sb[:],
                            start=(c == 0),
                            stop=(c == 3),
                        )
                        u2_ps = pnps.tile([128, cw], F32, tag="u")
                        nc.tensor.matmul(
                            out=u2_ps[:], lhsT=w1s_sb[:], rhs=rhs, start=True, stop=True
                        )
                        pr2_sb = pn.tile([128, cw], BF16, tag="pr")
                        nc.vector.tensor_tensor(
                            out=pr2_sb[:], in0=u2_ps[:], in1=atr_sb[:], op=mult
                        )
                        nc.tensor.matmul(
                            out=sT_ps[:],
                            lhsT=ssel_sb[:, c * 32 : (c + 1) * 32],
                            rhs=pr2_sb[:],
                            start=(c == 0),
                            stop=(c == 3),
                        )
                    nc.scalar.copy(out=sT_sb[:, base : base + cw], in_=sT_ps[:])
                    xT_sb = pn.tile([32, cw], F32, tag="xTs")
                    nc.scalar.copy(out=xT_sb[:], in_=xT_ps[:])
                    for q in range(cw // 128):
                        xr_ps = pnps.tile([128, 32], F32, tag="xr")
                        nc.tensor.transpose(
                            out=xr_ps[:],
                            in_=xT_sb[:, q * 128 : (q + 1) * 128],
                            identity=ident[:32, :32],
                        )
                        xr_sb = pn.tile([128, 32], F32, tag="xrs")
                        nc.scalar.copy(out=xr_sb[:], in_=xr_ps[:])
                        nc.sync.dma_start(
                            out=x_shard[base + q * 128 : base + (q + 1) * 128, :],
                            in_=xr_sb[:],
                        )

            # ---------------- allgather x ----------------
            nc.gpsimd.collective_compute(
                "AllGather",
                mybir.AluOpType.bypass,
                ins=[x_shard[:]],
                outs=[x_full[:]],
                replica_groups=[list(range(NCORES))],
            )

            # ---------------- phase E: edges ----------------
            with (
                tc.tile_pool(name="pe", bufs=3) as pe,
                tc.tile_pool(name="peps", bufs=2, space="PSUM") as peps,
            ):
                for b in range(nb):
                    efT_sb = pe.tile([32, T * 128], BF16, tag="efT")
                    nc.sync.dma_start(
                        out=efT_sb[:],
                        in_=d_b32[:, OEF + b * T * 128 : OEF + (b + 1) * T * 128],
                    )
                    # one-hot scatter matrices for the whole bin in one DVE op
                    oh_sb = pe.tile([128, T * 128], BF16, tag="oh")
                    nc.vector.tensor_tensor(
                        out=oh_sb[:].rearrange("p (t f) -> p t f", t=T),
                        in0=iota_sb[:].unsqueeze(1).to_broadcast((128, T, 128)),
                        in1=dstoff_sb[:, b * T : (b + 1) * T]
                        .unsqueeze(2)
                        .to_broadcast((128, T, 128)),
                        op=mybir.AluOpType.is_equal,
                    )
                    bin_ps = peps.tile([128, 256], F32, tag="bin")
                    for j in range(T):
                        t = b * T + j
                        # radial MLP layer 1 (bias via rank-1 matmul)
                        hT_ps = peps.tile([64, 128], F32, tag="hT")
                        nc.tensor.matmul(
                            out=hT_ps[:],
                            lhsT=b1_sb[:],
                            rhs=ones_sb[:],
                            start=True,
                            stop=False,
                        )
                        nc.tensor.matmul(
                            out=hT_ps[:],
                            lhsT=w1_sb[:],
                            rhs=efT_sb[:, j * 128 : (j + 1) * 128],
                            start=False,
                            stop=True,
                        )
                        ha_sb = pe.tile([64, 128], BF16, tag="ha")
                        nc.scalar.activation(
                            out=ha_sb[:],
                            in_=hT_ps[:],
                            func=mybir.ActivationFunctionType.Silu,
                        )
                        # layer 2 -> ef [128e, (i,o,d)], bias via rank-1 matmul
                        ef_ps = peps.tile([128, 512], F32, tag="ef")
                        nc.tensor.matmul(
                            out=ef_ps[:],
                            lhsT=ha_sb[:],
                            rhs=w2p64_sb[:],
                            start=True,
                            stop=False,
                        )
                        nc.tensor.matmul(
                            out=ef_ps[:],
                            lhsT=ones_sb[:],
                            rhs=b2p_sb[:],
                            start=False,
                            stop=True,
                        )
                        ef_sb = pe.tile([128, 512], BF16, tag="efs")
                        nc.scalar.copy(out=ef_sb[:], in_=ef_ps[:])
                        # gather x[src]
                        xg_sb = pe.tile([128, 32], F32, tag="xg")
                        nc.gpsimd.indirect_dma_start(
                            out=xg_sb[:],
                            out_offset=None,
                            in_=x_full[:],
                            in_offset=IndirectOffsetOnAxis(
                                ap=esrc_sb[:, t : t + 1], axis=0
                            ),
                        )
                        # bilinear message, all-flat APs:
                        # msg[e,(c,o,d)] = sum_i xg[e,(c,i)] * ef[e,(i,(o,d))]
                        msg_sb = pe.tile([128, 256], F32, tag="msg")
                        msgb_sb = pe.tile([128, 256], BF16, tag="msgb")
                        for c in range(4):
                            eng = nc.vector
                            mslice = msg_sb[:, c * 64 : (c + 1) * 64]
                            for i in range(8):
                                x_ci = xg_sb[:, c * 8 + i : c * 8 + i + 1]
                                ef_i = ef_sb[:, i * 64 : (i + 1) * 64]
                                if i == 0:
                                    eng.tensor_scalar_mul(
                                        out=mslice, in0=ef_i, scalar1=x_ci
                                    )
                                else:
                                    out_ap = (
                                        msgb_sb[:, c * 64 : (c + 1) * 64]
                                        if i == 7
                                        else mslice
                                    )
                                    eng.scalar_tensor_tensor(
                                        out=out_ap,
                                        in0=ef_i,
                                        scalar=x_ci,
                                        in1=mslice,
                                        op0=mult,
                                        op1=addop,
                                    )
                        nc.tensor.matmul(
                            out=bin_ps[:],
                            lhsT=oh_sb[:, j * 128 : (j + 1) * 128],
                            rhs=msgb_sb[:],
                            start=(j == 0),
                            stop=(j == T - 1),
                        )
                    nc.scalar.copy(
                        out=slab[:, b * 256 : (b + 1) * 256].rearrange(
                            "p (o c d) -> p c o d", o=8, c=4
                        ),
                        in_=bin_ps[:].rearrange("p (c o d) -> p c o d", o=8, c=4),
                    )

            # ---------------- phase F: lin2 + self-connection ----------------
            with (
                tc.tile_pool(name="pf", bufs=3) as pf,
                tc.tile_pool(name="pfps", bufs=2, space="PSUM") as pfps,
            ):
                for b in range(nb):
                    a2t_ps = pfps.tile([64, 128], F32, tag="a2t")
                    nc.tensor.matmul(
                        out=a2t_ps[:],
                        lhsT=w2lr_sb[:],
                        rhs=attrT_sb[:, b * 128 : (b + 1) * 128],
                        start=True,
                        stop=True,
                    )
                    a2t_sb = pf.tile([64, 128], F32, tag="a2ts")
                    nc.scalar.copy(out=a2t_sb[:], in_=a2t_ps[:])
                    a2_ps = pfps.tile([128, 64], F32, tag="a2")
                    nc.tensor.transpose(
                        out=a2_ps[:], in_=a2t_sb[:], identity=ident[:64, :64]
                    )
                    a2_sb = pf.tile([128, 64], F32, tag="a2s")
                    nc.scalar.copy(out=a2_sb[:], in_=a2_ps[:])

                    # x2[n,(p,c,d)] = sum_o A2[n,(o,p)] * slab[n,(c,o,d)]
                    x2_sb = pf.tile([128, 256], F32, tag="x2")
                    slab_b = slab[:, b * 256 : (b + 1) * 256]
                    for p in range(8):
                        eng = nc.vector
                        x2p = x2_sb[:, p * 32 : (p + 1) * 32]
                        for o in range(8):
                            a2_op = a2_sb[:, o * 8 + p : o * 8 + p + 1]
                            ag_o = slab_b[:, o * 32 : (o + 1) * 32]
                            if o == 0:
                                eng.tensor_scalar_mul(
                                    out=x2p, in0=ag_o, scalar1=a2_op
                                )
                            else:
                                eng.scalar_tensor_tensor(
                                    out=x2p,
                                    in0=ag_o,
                                    scalar=a2_op,
                                    in1=x2p,
                                    op0=mult,
                                    op1=addop,
                                )
                    s_ps = pfps.tile([128, 32], F32, tag="s")
                    nc.tensor.transpose(
                        out=s_ps[:],
                        in_=sT_sb[:, b * 128 : (b + 1) * 128],
                        identity=ident[:32, :32],
                    )
                    out_sb = pf.tile([128, 256], BF16, tag="outt")
                    # out[n,(p,c,d)] = x2 + s[n,(c,p)] broadcast over d
                    s_b = (
                        s_ps[:]
                        .rearrange("p (c o) -> p o c", o=8)
                        .unsqueeze(3)
                        .to_broadcast((128, 8, 4, 8))
                    )
                    x2_r = x2_sb[:].rearrange("p (q c d) -> p q c d", c=4, d=8)
                    out_r = out_sb[:].rearrange("p (q c d) -> p q c d", c=4, d=8)
                    nc.vector.tensor_tensor(out=out_r, in0=x2_r, in1=s_b, op=addop)
                    nc.sync.dma_start(
                        out=d_out[b * 128 : (b + 1) * 128, :], in_=out_sb[:]
                    )

    nc.finalize()
    return nc


_BUILD_CACHE = {}


def kernel(**inputs):
    n = inputs["node_input"].shape[0]
    ns = n // NCORES
    nbin = 128
    in_maps, T, nb, npad, s_total = _host_prep(inputs, ns, nbin)
    key = (T, nb, npad, s_total, ns)
    if key not in _BUILD_CACHE:
        _BUILD_CACHE[key] = _build(T, nb, npad, s_total, ns)
    nc = _BUILD_CACHE[key]
    res = run_bass_kernel_spmd(nc, in_maps, list(range(NCORES)))
    # device rows are (p, c, d) in bf16; reference output is (c, d, p) f32
    out = np.concatenate(
        [
            np.asarray(res.results[k]["out"][:ns], np.float32)
            .reshape(ns, 8, 4, 8)
            .transpose(0, 2, 3, 1)
            for k in range(NCORES)
        ],
        axis=0,
    )
    return np.ascontiguousarray(out, np.float32)
